# revision 12
# baseline (speedup 1.0000x reference)
"""KMaxPooling (top-8 along seq axis) Bass kernel for TRN2, 8-core SPMD.

Input  x: (64, 4096, 256) fp32. Output: (64, 8, 256) fp32 = per (batch,
channel) the 8 largest values over the 4096 seq positions, descending.

Strategy (per core, batch-sharded 8 ways -> 8 batches/core, 32 MB):
  - one 4 MB contiguous DMA per batch into SBUF (seq%128 -> partition)
  - PE transposes 128x128 blocks into PSUM so channels land on partitions
  - DVE InstMax (hardware top-8, sorted desc) over 2048-wide PSUM spans
  - tiny second-level InstMax merges the two half-candidates
  - one 64 KB output DMA per core; host reassembles pure layout
"""

import sys

sys.path.insert(0, "/opt/trn_rl_repo")

import numpy as np

import concourse.bass as bass
import concourse.mybir as mybir
from concourse import masks
from concourse.tile import TileContext
from concourse.vector_clock import ScopedClock, VectorClock
from concourse.bass_utils import run_bass_kernel_spmd

B, S, C, K = 64, 4096, 256, 8
NCORES = 8
BPC = B // NCORES  # batches per core
SEQ_TILES = S // 128  # 32
CH_GROUPS = C // 128  # 2
HALF_TILES = SEQ_TILES // 2  # 16 seq tiles per PSUM fill (4 banks)

F32 = mybir.dt.float32

N_PROCS = 27


class SplitDrainTileContext(TileContext):
    """The walrus backend here rejects any instruction carrying more than
    one sync wait ("Too many sync wait commands"), but Tile's semaphore
    assignment can attach several. Two fixes:

    1. _lower_ordered_insts: before lowering, hoist excess waits of every
       scheduled instruction onto single-wait same-engine NoOps inserted
       right before it.
    2. _drain_and_barrier: emit one single-wait drain per logical proc
       instead of one drain waiting on the whole global vector clock.
    """

    def _lower_ordered_insts(self, ordered):
        for bb_name, insts in ordered.items():
            rewritten = []
            for inst in insts:
                si = inst.sync_info
                if si is not None and si.on_wait and len(si.on_wait) > 1:
                    waits = list(si.on_wait)
                    for k, w in enumerate(waits[:-1]):
                        nop = mybir.InstNoOp(
                            name=f"{inst.name}.wsplit{k}",
                            engine=inst.engine,
                            sync_info=mybir.SyncInfo(on_wait=[w], on_update=[]),
                            bass_nofuse=True,
                        )
                        rewritten.append(nop)
                    si.on_wait = waits[-1:]
                rewritten.append(inst)
            ordered[bb_name] = rewritten
        return super()._lower_ordered_insts(ordered)

    def _drain_and_barrier(self, tick_clock, wait_clock):
        gc = tick_clock.global_clock
        for p in range(N_PROCS):
            if gc[p] > 0:
                v = [0] * N_PROCS
                v[p] = gc[p]
                di = self.nc.sync.drain()
                wait_clock.add_sem_waits(di.ins, ScopedClock({None: VectorClock(v)}))

        self.nc.all_engine_barrier()
        assert self.sems is not None
        popped = self.nc._tile_sem_poison_stack.pop()
        assert popped is self._sem_poison
        self.nc.clear_and_free_semaphores(list(self.sems.allocated().values()))
        self.nc.all_engine_barrier()


def build_program():
    nc = bass.Bass()
    x_ext = nc.declare_dram_parameter("x", [BPC, S, C], F32, isOutput=False)
    # out[c', g*64 + b*8 + k]: top-k values of channel g*128+c' in batch b
    out_ext = nc.declare_dram_parameter(
        "out", [128, CH_GROUPS * BPC * K], F32, isOutput=True
    )

    with SplitDrainTileContext(nc) as tc:
        with (
            tc.tile_pool(name="const", bufs=1) as const_pool,
            tc.tile_pool(name="xin", bufs=6) as in_pool,
            tc.tile_pool(name="psum", bufs=2, space="PSUM") as psum_pool,
            tc.tile_pool(name="cand", bufs=4) as cand_pool,
            tc.tile_pool(name="obuf", bufs=1) as out_pool,
        ):
            # Build identity in fp32 (gpsimd can't memset fp32r), then
            # DMA-copy the bits into an fp32r tile: the BIR verifier wants
            # fp32r matmult inputs produced as fp32r.
            identity_f32 = const_pool.tile([128, 128], F32)
            masks.make_identity(nc, identity_f32[:])
            identity = const_pool.tile([128, 128], mybir.dt.float32r)
            nc.gpsimd.dma_start(out=identity[:], in_=identity_f32[:].bitcast(mybir.dt.float32r))

            obuf = out_pool.tile([128, CH_GROUPS * BPC * K], F32)

            # seq halves per batch: (b, h) -> 2 MB loads with one contiguous
            # 16 KB chunk per partition (one big DMA descriptor each, which
            # amortizes the ~70ns/descriptor engine overhead). One 4-bank
            # PSUM span covers one half load so InstMax stays at 2048-wide
            # calls. Alternate SP/Act HWDGE rings.
            F32R = mybir.dt.float32r
            HT = HALF_TILES  # 16 seq tiles per half load
            dma_engines = [nc.sync, nc.scalar]
            cands = {}
            for b in range(BPC):
                for h in range(2):
                    # fp32r tile (bit-identical to fp32) so the verifier
                    # accepts it as an fp32r transpose-matmult input.
                    xin = in_pool.tile([128, HT * C], F32R)
                    # xin[p, j*C + c] = x[b, seq_lo + p*HT + j, c]; top-k is
                    # order-invariant over seq so any permutation works.
                    seq_lo = h * HT * 128
                    seq_hi = (h + 1) * HT * 128
                    dma_engines[h].dma_start(
                        out=xin[:],
                        in_=x_ext[b, seq_lo:seq_hi]
                        .rearrange("(p t) c -> p t c", p=128)
                        .bitcast(F32R),
                    )
                    last_b = b == BPC - 1
                    for g in range(CH_GROUPS):
                        if h == 0:
                            nslots = 3 * K if last_b else 2 * K
                            cands[(b, g)] = cand_pool.tile(
                                [128, nslots], F32, name="cand", tag="cand"
                            )
                        cand = cands[(b, g)]
                        ps = psum_pool.tile([128, HT * 128], F32, name="ps", tag="ps")
                        for j in range(HT):
                            col = j * C + g * 128
                            # fp32r transpose: 1.5 cycles/row vs fp32's 2.0
                            # (pure data movement; fp32r rounds ~tf32).
                            nc.tensor.matmul(
                                ps[:, 128 * j : 128 * (j + 1)].bitcast(F32R),
                                xin[:, col : col + 128],
                                identity[:],
                                is_transpose=True,
                                start=True,
                                stop=True,
                            )
                        if last_b and h == 1:
                            # split the final span's InstMax so the first
                            # 1024 overlaps the PE finishing the second
                            nc.vector.max(out=cand[:, K : 2 * K], in_=ps[:, 0:1024])
                            nc.vector.max(
                                out=cand[:, 2 * K : 3 * K], in_=ps[:, 1024:2048]
                            )
                        else:
                            nc.vector.max(out=cand[:, K * h : K * (h + 1)], in_=ps[:])
                        if h == 1:
                            nc.vector.max(
                                out=obuf[
                                    :, (g * BPC + b) * K : (g * BPC + b + 1) * K
                                ],
                                in_=cand[:],
                            )

            nc.sync.dma_start(out=out_ext[:], in_=obuf[:])

    return nc


_prog = None


def _get_prog():
    global _prog
    if _prog is None:
        _prog = build_program()
    return _prog


def run_on_cores(x: np.ndarray, **run_kwargs):
    """Shard, run on 8 cores, return (full_output, BassKernelResults)."""
    nc = _get_prog()
    x = np.ascontiguousarray(np.asarray(x, dtype=np.float32))
    in_maps = [
        {"x": np.ascontiguousarray(x[i * BPC : (i + 1) * BPC])} for i in range(NCORES)
    ]
    res = run_bass_kernel_spmd(nc, in_maps, list(range(NCORES)), **run_kwargs)
    parts = []
    for i in range(NCORES):
        o = res.results[i]["out"]  # (128, CH_GROUPS*BPC*K)
        o = o.reshape(128, CH_GROUPS, BPC, K)  # (c', g, b, k)
        o = o.transpose(2, 3, 1, 0).reshape(BPC, K, C)  # (b, k, g*128+c')
        parts.append(o)
    return np.concatenate(parts, axis=0), res


def kernel(x: np.ndarray) -> np.ndarray:
    out, _ = run_on_cores(x)
    return out



# revision 14
# speedup vs baseline: 1.0122x; 1.0122x over previous
"""KMaxPooling (top-8 along seq axis) Bass kernel for TRN2, 8-core SPMD.

Input  x: (64, 4096, 256) fp32. Output: (64, 8, 256) fp32 = per (batch,
channel) the 8 largest values over the 4096 seq positions, descending.

Strategy (per core, batch-sharded 8 ways -> 8 batches/core, 32 MB):
  - one 4 MB contiguous DMA per batch into SBUF (seq%128 -> partition)
  - PE transposes 128x128 blocks into PSUM so channels land on partitions
  - DVE InstMax (hardware top-8, sorted desc) over 2048-wide PSUM spans
  - tiny second-level InstMax merges the two half-candidates
  - one 64 KB output DMA per core; host reassembles pure layout
"""

import sys

sys.path.insert(0, "/opt/trn_rl_repo")

import numpy as np

import concourse.bass as bass
import concourse.mybir as mybir
from concourse import masks
from concourse.tile import TileContext
from concourse.vector_clock import ScopedClock, VectorClock
from concourse.bass_utils import run_bass_kernel_spmd

B, S, C, K = 64, 4096, 256, 8
NCORES = 8
BPC = B // NCORES  # batches per core
SEQ_TILES = S // 128  # 32
CH_GROUPS = C // 128  # 2
HALF_TILES = SEQ_TILES // 2  # 16 seq tiles per PSUM fill (4 banks)

F32 = mybir.dt.float32

N_PROCS = 27


class SplitDrainTileContext(TileContext):
    """The walrus backend here rejects any instruction carrying more than
    one sync wait ("Too many sync wait commands"), but Tile's semaphore
    assignment can attach several. Two fixes:

    1. _lower_ordered_insts: before lowering, hoist excess waits of every
       scheduled instruction onto single-wait same-engine NoOps inserted
       right before it.
    2. _drain_and_barrier: emit one single-wait drain per logical proc
       instead of one drain waiting on the whole global vector clock.
    """

    def _lower_ordered_insts(self, ordered):
        for bb_name, insts in ordered.items():
            rewritten = []
            for inst in insts:
                si = inst.sync_info
                if si is not None and si.on_wait and len(si.on_wait) > 1:
                    waits = list(si.on_wait)
                    for k, w in enumerate(waits[:-1]):
                        nop = mybir.InstNoOp(
                            name=f"{inst.name}.wsplit{k}",
                            engine=inst.engine,
                            sync_info=mybir.SyncInfo(on_wait=[w], on_update=[]),
                            bass_nofuse=True,
                        )
                        rewritten.append(nop)
                    si.on_wait = waits[-1:]
                rewritten.append(inst)
            ordered[bb_name] = rewritten
        return super()._lower_ordered_insts(ordered)

    def _drain_and_barrier(self, tick_clock, wait_clock):
        gc = tick_clock.global_clock
        for p in range(N_PROCS):
            if gc[p] > 0:
                v = [0] * N_PROCS
                v[p] = gc[p]
                di = self.nc.sync.drain()
                wait_clock.add_sem_waits(di.ins, ScopedClock({None: VectorClock(v)}))

        self.nc.all_engine_barrier()
        assert self.sems is not None
        popped = self.nc._tile_sem_poison_stack.pop()
        assert popped is self._sem_poison
        self.nc.clear_and_free_semaphores(list(self.sems.allocated().values()))
        self.nc.all_engine_barrier()


def build_program():
    nc = bass.Bass()
    x_ext = nc.declare_dram_parameter("x", [BPC, S, C], F32, isOutput=False)
    # out[c', g*64 + b*8 + k]: top-k values of channel g*128+c' in batch b
    out_ext = nc.declare_dram_parameter(
        "out", [128, CH_GROUPS * BPC * K], F32, isOutput=True
    )

    with SplitDrainTileContext(nc) as tc:
        with (
            tc.tile_pool(name="const", bufs=1) as const_pool,
            tc.tile_pool(name="xin", bufs=8) as in_pool,
            tc.tile_pool(name="psum", bufs=2, space="PSUM") as psum_pool,
            tc.tile_pool(name="cand", bufs=4) as cand_pool,
            tc.tile_pool(name="obuf", bufs=1) as out_pool,
        ):
            # Build identity in fp32 (gpsimd can't memset fp32r), then
            # round-copy it into an fp32r tile on the Activation engine:
            # the BIR verifier wants fp32r matmult inputs produced as
            # fp32r, and an SBUF->SBUF DMA here would trickle behind the
            # big input loads on the shared DMA engines.
            identity_f32 = const_pool.tile([128, 128], F32)
            masks.make_identity(nc, identity_f32[:])
            identity = const_pool.tile([128, 128], mybir.dt.float32r)
            nc.scalar.copy(out=identity[:], in_=identity_f32[:])

            obuf = out_pool.tile([128, CH_GROUPS * BPC * K], F32)

            # Each half-batch (2 MB, one 2048-wide PSUM span per channel
            # group) is loaded as two 1 MB quarters issued on BOTH HWDGE
            # rings in parallel, 8 KB contiguous per partition per
            # descriptor. Parallel rings halve the time-to-first-data and
            # deliver a steady half every ~5us, keeping the DVE (the
            # compute floor) continuously fed. The very first half is
            # split into 0.5 MB eighths so the PE starts ~2us sooner.
            F32R = mybir.dt.float32r
            HT = HALF_TILES  # 16 seq tiles per half span
            dma_engines = [nc.sync, nc.scalar]
            cands = {}
            for b in range(BPC):
                for h in range(2):
                    pieces = 4 if (b == 0 and h == 0) else 2
                    tpp = HT // pieces  # seq tiles per DMA piece
                    xins = []
                    for pc in range(pieces):
                        xin = in_pool.tile([128, tpp * C], F32R)
                        # xin[p, t*C + c] = x[b, piece_lo + p*tpp + t, c];
                        # top-k is order-invariant over seq so any
                        # permutation works.
                        seq_lo = (h * HT + pc * tpp) * 128
                        seq_hi = seq_lo + tpp * 128
                        dma_engines[pc % 2].dma_start(
                            out=xin[:],
                            in_=x_ext[b, seq_lo:seq_hi]
                            .rearrange("(p t) c -> p t c", p=128)
                            .bitcast(F32R),
                        )
                        xins.append(xin)
                    last_b = b == BPC - 1
                    for g in range(CH_GROUPS):
                        if h == 0:
                            nslots = 3 * K if last_b else 2 * K
                            cands[(b, g)] = cand_pool.tile(
                                [128, nslots], F32, name="cand", tag="cand"
                            )
                        cand = cands[(b, g)]
                        ps = psum_pool.tile([128, HT * 128], F32, name="ps", tag="ps")
                        for j in range(HT):
                            col = (j % tpp) * C + g * 128
                            # fp32r transpose: 1.5 cycles/row vs fp32's 2.0
                            # (pure data movement; fp32r rounds ~tf32).
                            nc.tensor.matmul(
                                ps[:, 128 * j : 128 * (j + 1)].bitcast(F32R),
                                xins[j // tpp][:, col : col + 128],
                                identity[:],
                                is_transpose=True,
                                start=True,
                                stop=True,
                            )
                        if last_b and h == 1:
                            # split the final span's InstMax so the first
                            # 1024 overlaps the PE finishing the second
                            nc.vector.max(out=cand[:, K : 2 * K], in_=ps[:, 0:1024])
                            nc.vector.max(
                                out=cand[:, 2 * K : 3 * K], in_=ps[:, 1024:2048]
                            )
                        else:
                            nc.vector.max(out=cand[:, K * h : K * (h + 1)], in_=ps[:])
                        if h == 1:
                            nc.vector.max(
                                out=obuf[
                                    :, (g * BPC + b) * K : (g * BPC + b + 1) * K
                                ],
                                in_=cand[:],
                            )

            nc.sync.dma_start(out=out_ext[:], in_=obuf[:])

    return nc


_prog = None


def _get_prog():
    global _prog
    if _prog is None:
        _prog = build_program()
    return _prog


def run_on_cores(x: np.ndarray, **run_kwargs):
    """Shard, run on 8 cores, return (full_output, BassKernelResults)."""
    nc = _get_prog()
    x = np.ascontiguousarray(np.asarray(x, dtype=np.float32))
    in_maps = [
        {"x": np.ascontiguousarray(x[i * BPC : (i + 1) * BPC])} for i in range(NCORES)
    ]
    res = run_bass_kernel_spmd(nc, in_maps, list(range(NCORES)), **run_kwargs)
    parts = []
    for i in range(NCORES):
        o = res.results[i]["out"]  # (128, CH_GROUPS*BPC*K)
        o = o.reshape(128, CH_GROUPS, BPC, K)  # (c', g, b, k)
        o = o.transpose(2, 3, 1, 0).reshape(BPC, K, C)  # (b, k, g*128+c')
        parts.append(o)
    return np.concatenate(parts, axis=0), res


def kernel(x: np.ndarray) -> np.ndarray:
    out, _ = run_on_cores(x)
    return out



# revision 20
# speedup vs baseline: 1.0268x; 1.0144x over previous
"""KMaxPooling (top-8 along seq axis) Bass kernel for TRN2, 8-core SPMD.

Input  x: (64, 4096, 256) fp32. Output: (64, 8, 256) fp32 = per (batch,
channel) the 8 largest values over the 4096 seq positions, descending.

Strategy (per core, batch-sharded 8 ways -> 8 batches/core, 32 MB):
  - one 4 MB contiguous DMA per batch into SBUF (seq%128 -> partition)
  - PE transposes 128x128 blocks into PSUM so channels land on partitions
  - DVE InstMax (hardware top-8, sorted desc) over 2048-wide PSUM spans
  - tiny second-level InstMax merges the two half-candidates
  - one 64 KB output DMA per core; host reassembles pure layout
"""

import sys

sys.path.insert(0, "/opt/trn_rl_repo")

import numpy as np

import concourse.bass as bass
import concourse.mybir as mybir
from concourse import masks
from concourse.tile import TileContext
from concourse.vector_clock import ScopedClock, VectorClock
from concourse.bass_utils import run_bass_kernel_spmd

B, S, C, K = 64, 4096, 256, 8
NCORES = 8
BPC = B // NCORES  # batches per core
SEQ_TILES = S // 128  # 32
CH_GROUPS = C // 128  # 2
HALF_TILES = SEQ_TILES // 2  # 16 seq tiles per PSUM fill (4 banks)

F32 = mybir.dt.float32

N_PROCS = 27


class SplitDrainTileContext(TileContext):
    """The walrus backend here rejects any instruction carrying more than
    one sync wait ("Too many sync wait commands"), but Tile's semaphore
    assignment can attach several. Two fixes:

    1. _lower_ordered_insts: before lowering, hoist excess waits of every
       scheduled instruction onto single-wait same-engine NoOps inserted
       right before it.
    2. _drain_and_barrier: emit one single-wait drain per logical proc
       instead of one drain waiting on the whole global vector clock.
    """

    def _lower_ordered_insts(self, ordered):
        for bb_name, insts in ordered.items():
            rewritten = []
            for inst in insts:
                si = inst.sync_info
                if si is not None and si.on_wait and len(si.on_wait) > 1:
                    waits = list(si.on_wait)
                    for k, w in enumerate(waits[:-1]):
                        nop = mybir.InstNoOp(
                            name=f"{inst.name}.wsplit{k}",
                            engine=inst.engine,
                            sync_info=mybir.SyncInfo(on_wait=[w], on_update=[]),
                            bass_nofuse=True,
                        )
                        rewritten.append(nop)
                    si.on_wait = waits[-1:]
                rewritten.append(inst)
            ordered[bb_name] = rewritten
        return super()._lower_ordered_insts(ordered)

    def _drain_and_barrier(self, tick_clock, wait_clock):
        gc = tick_clock.global_clock
        for p in range(N_PROCS):
            if gc[p] > 0:
                v = [0] * N_PROCS
                v[p] = gc[p]
                di = self.nc.sync.drain()
                wait_clock.add_sem_waits(di.ins, ScopedClock({None: VectorClock(v)}))

        self.nc.all_engine_barrier()
        assert self.sems is not None
        popped = self.nc._tile_sem_poison_stack.pop()
        assert popped is self._sem_poison
        self.nc.clear_and_free_semaphores(list(self.sems.allocated().values()))
        self.nc.all_engine_barrier()


def build_program():
    nc = bass.Bass()
    x_ext = nc.declare_dram_parameter("x", [BPC, S, C], F32, isOutput=False)
    # out[c', g*64 + b*8 + k]: top-k values of channel g*128+c' in batch b
    out_ext = nc.declare_dram_parameter(
        "out", [128, CH_GROUPS * BPC * K], F32, isOutput=True
    )

    with SplitDrainTileContext(nc) as tc:
        with (
            tc.tile_pool(name="const", bufs=1) as const_pool,
            tc.tile_pool(name="xin", bufs=16) as in_pool,
            tc.tile_pool(name="xin8", bufs=4) as in8_pool,
            tc.tile_pool(name="psum", bufs=2, space="PSUM") as psum_pool,
            tc.tile_pool(name="cand", bufs=4) as cand_pool,
            tc.tile_pool(name="obuf", bufs=1) as out_pool,
        ):
            # Build identity in fp32 (gpsimd can't memset fp32r), then
            # round-copy it into an fp32r tile on the Activation engine:
            # the BIR verifier wants fp32r matmult inputs produced as
            # fp32r, and an SBUF->SBUF DMA here would trickle behind the
            # big input loads on the shared DMA engines.
            identity_f32 = const_pool.tile([128, 128], F32)
            masks.make_identity(nc, identity_f32[:])
            identity = const_pool.tile([128, 128], mybir.dt.float32r)
            nc.scalar.copy(out=identity[:], in_=identity_f32[:])

            obuf = out_pool.tile([128, CH_GROUPS * BPC * K], F32)

            # Warm up the PE's DVFS ramp while the first loads are in
            # flight: transposes of the identity into a throwaway PSUM
            # span. Nothing reads it; the pool reuses the buffer later
            # with only a PE-serial write-after-write dependency.
            F32R = mybir.dt.float32r
            warm = psum_pool.tile([128, HALF_TILES * 128], F32, name="ps", tag="ps")
            for _ in range(8):
                nc.tensor.matmul(
                    warm[:, 0:128].bitcast(F32R),
                    identity[:],
                    identity[:],
                    is_transpose=True,
                    start=True,
                    stop=True,
                )

            # Each half-batch (2 MB, one 2048-wide PSUM span per channel
            # group) is loaded as two 1 MB quarters issued on BOTH HWDGE
            # rings in parallel, 8 KB contiguous per partition per
            # descriptor. Parallel rings halve the time-to-first-data and
            # deliver a steady half every ~5us, keeping the DVE (the
            # compute floor) continuously fed. The very first half is
            # split into 0.5 MB eighths so the PE starts ~2us sooner.
            HT = HALF_TILES  # 16 seq tiles per half span
            dma_engines = [nc.sync, nc.scalar]
            cands = {}
            for b in range(BPC):
                for h in range(2):
                    pieces = 4 if (b == 0 and h == 0) else 2
                    tpp = HT // pieces  # seq tiles per DMA piece
                    pool = in8_pool if pieces == 4 else in_pool
                    xins = []
                    for pc in range(pieces):
                        xin = pool.tile([128, tpp * C], F32R)
                        # xin[p, t*C + c] = x[b, piece_lo + p*tpp + t, c];
                        # top-k is order-invariant over seq so any
                        # permutation works.
                        seq_lo = (h * HT + pc * tpp) * 128
                        seq_hi = seq_lo + tpp * 128
                        dma_engines[pc % 2].dma_start(
                            out=xin[:],
                            in_=x_ext[b, seq_lo:seq_hi]
                            .rearrange("(p t) c -> p t c", p=128)
                            .bitcast(F32R),
                        )
                        xins.append(xin)
                    last_b = b == BPC - 1
                    for g in range(CH_GROUPS):
                        if h == 0:
                            nslots = 3 * K if (last_b or b == 0) else 2 * K
                            cands[(b, g)] = cand_pool.tile(
                                [128, nslots], F32, name="cand", tag="cand"
                            )
                        cand = cands[(b, g)]
                        ps = psum_pool.tile([128, HT * 128], F32, name="ps", tag="ps")
                        for j in range(HT):
                            col = (j % tpp) * C + g * 128
                            # fp32r transpose: 1.5 cycles/row vs fp32's 2.0
                            # (pure data movement; fp32r rounds ~tf32).
                            nc.tensor.matmul(
                                ps[:, 128 * j : 128 * (j + 1)].bitcast(F32R),
                                xins[j // tpp][:, col : col + 128],
                                identity[:],
                                is_transpose=True,
                                start=True,
                                stop=True,
                            )
                        # First and last spans split their InstMax in two
                        # 1024-wide calls so the DVE starts/finishes while
                        # the PE is still filling the other half.
                        if b == 0 and h == 0:
                            nc.vector.max(out=cand[:, 0:K], in_=ps[:, 0:1024])
                            nc.vector.max(out=cand[:, K : 2 * K], in_=ps[:, 1024:2048])
                        elif last_b and h == 1:
                            nc.vector.max(out=cand[:, K : 2 * K], in_=ps[:, 0:1024])
                            nc.vector.max(
                                out=cand[:, 2 * K : 3 * K], in_=ps[:, 1024:2048]
                            )
                        elif b == 0:
                            nc.vector.max(out=cand[:, 2 * K : 3 * K], in_=ps[:])
                        else:
                            nc.vector.max(out=cand[:, K * h : K * (h + 1)], in_=ps[:])
                        if h == 1:
                            nc.vector.max(
                                out=obuf[
                                    :, (g * BPC + b) * K : (g * BPC + b + 1) * K
                                ],
                                in_=cand[:],
                            )

            nc.sync.dma_start(out=out_ext[:], in_=obuf[:])

    return nc


_prog = None


def _get_prog():
    global _prog
    if _prog is None:
        _prog = build_program()
    return _prog


def run_on_cores(x: np.ndarray, **run_kwargs):
    """Shard, run on 8 cores, return (full_output, BassKernelResults)."""
    nc = _get_prog()
    x = np.ascontiguousarray(np.asarray(x, dtype=np.float32))
    in_maps = [
        {"x": np.ascontiguousarray(x[i * BPC : (i + 1) * BPC])} for i in range(NCORES)
    ]
    res = run_bass_kernel_spmd(nc, in_maps, list(range(NCORES)), **run_kwargs)
    parts = []
    for i in range(NCORES):
        o = res.results[i]["out"]  # (128, CH_GROUPS*BPC*K)
        o = o.reshape(128, CH_GROUPS, BPC, K)  # (c', g, b, k)
        o = o.transpose(2, 3, 1, 0).reshape(BPC, K, C)  # (b, k, g*128+c')
        parts.append(o)
    return np.concatenate(parts, axis=0), res


def kernel(x: np.ndarray) -> np.ndarray:
    out, _ = run_on_cores(x)
    return out

